# revision 3
# baseline (speedup 1.0000x reference)
"""Single-head causal attention (B=4, S=4096, D=1024, H=64) on 8 TRN2 NeuronCores.

v2: quarter-pipelined, causal-trimmed, queue-disciplined.

Sharding: 2 cores per batch; core j of the pair takes q-blocks of parity j
(interleaved 128-row blocks balance causal work). Odd cores get x's kv
128-block pairs swapped host-side so the instruction stream is identical
(SPMD); the residual structural difference (odd-chunk leading block fully
masked for j=0, fully kept for j=1) is absorbed into a per-core mask tile.

Device algorithm per core (bf16 matmuls, f32 PSUM):
  per s-quarter qq (1024 kv positions, 512 q columns):
    K^T|V^T += [Wk|Wv]^T @ x^T   (8 d-chunks, 2x512-col segments)
    Q^T     = Wq^T @ x^T          (e=0 block-parity columns only)
    k2      = odd chunks of K^T copied to partitions 64-127 (row tiling)
    vn      = V natural per 128-chunk via DMA transpose, ones col appended
    group t=qq: for kv chunk c in 0..8t+7:
      piece = S^T(c, q in [max(512t, 128*(c//2)), 512(t+1)))  -- causal trim
      scores matmul into packed (128,1024) PSUM tiles; even chunks use
      kvt rows 0-63, odd chunks k2 rows 64-127 (concurrent row-tiled PE)
      one exp ACT per PSUM tile (scale=1/8 fused)
      leading 128-col block of every chunk masked: even chunks x tril,
      odd chunks x per-core 0/1 tile
      PV: o_acc[65, 512] += [V|1]^T @ P^T  (row 64 = softmax denominator)
  out[t] = o_acc_t  (host divides rows 0-63 by row 64 and scatters)
"""

import sys

for _p in ("/opt/trn_rl_repo", "/root/.axon_site"):
    if _p not in sys.path:
        sys.path.insert(0, _p)

import numpy as np
import ml_dtypes

B, S, D, H = 4, 4096, 1024, 64
N_CORES = 8
DC = D // 128           # 8 d-chunks
NQ = 4                  # s-quarters
NKC = S // 128          # 32 kv 128-blocks
SCALE = 1.0 / 8.0

BF16 = ml_dtypes.bfloat16

_cached = {}

N_WARMUP = 16           # 128-col PE warmup matmuls (~3.5us busy to raise HAM)


def _build_nc():
    from concourse import bacc, tile, mybir
    from concourse.masks import make_identity

    f32 = mybir.dt.float32
    bf16 = mybir.dt.bfloat16

    nc = bacc.Bacc("TRN2", target_bir_lowering=False, debug=False,
                   num_devices=N_CORES)

    xT = nc.declare_dram_parameter("xT", [D, S], bf16, isOutput=False)
    wkv = nc.declare_dram_parameter("wkv", [128, DC, 128], bf16, isOutput=False)
    wq = nc.declare_dram_parameter("wq", [128, DC, H], bf16, isOutput=False)
    bkv = nc.declare_dram_parameter("bkv", [128, 1], f32, isOutput=False)
    bqp = nc.declare_dram_parameter("bq", [H, 1], f32, isOutput=False)
    tril = nc.declare_dram_parameter("tril", [128, 128], bf16, isOutput=False)
    msk2 = nc.declare_dram_parameter("msk2", [128, 128], bf16, isOutput=False)
    out = nc.declare_dram_parameter("out", [NQ, 65, 512], f32, isOutput=True)

    with tile.TileContext(nc) as tc:
        with (
            tc.tile_pool(name="consts", bufs=1) as consts,
            tc.tile_pool(name="xtp", bufs=1) as xtp,
            tc.tile_pool(name="kvp", bufs=1) as kvp,
            tc.tile_pool(name="ptp", bufs=3) as ptp,
            tc.tile_pool(name="osbp", bufs=2) as osbp,
            tc.tile_pool(name="pscore", bufs=2, space="PSUM") as pscore,
            tc.tile_pool(name="pacc", bufs=1, space="PSUM") as pacc,
        ):
            # ---- constants on the scalar queue (idle until first exp);
            # sync/gpsimd queues are reserved for x streaming ----
            tril_sb = consts.tile([128, 128], bf16)
            nc.scalar.dma_start(out=tril_sb[:], in_=tril[:, :])
            msk2_sb = consts.tile([128, 128], bf16)
            nc.scalar.dma_start(out=msk2_sb[:], in_=msk2[:, :])
            bkv_sb = consts.tile([128, 1], f32)
            nc.scalar.dma_start(out=bkv_sb[:], in_=bkv[:, :])
            bq_sb = consts.tile([H, 1], f32)
            nc.scalar.dma_start(out=bq_sb[:], in_=bqp[:, :])
            wq_sb = consts.tile([128, DC, H], bf16)
            nc.scalar.dma_start(out=wq_sb[:], in_=wq[:, :, :])
            wkv_sb = consts.tile([128, DC, 128], bf16)
            nc.scalar.dma_start(out=wkv_sb[:, 0:4, :], in_=wkv[:, 0:4, :])
            nc.scalar.dma_start(out=wkv_sb[:, 4:8, :], in_=wkv[:, 4:8, :])
            ident_bf = consts.tile([128, 128], bf16)
            make_identity(nc, ident_bf[:, :])

            # ---- persistent SBUF tensors ----
            kvt = kvp.tile([128, S], bf16)          # rows 0-63 K^T, 64-127 V^T
            k2 = kvp.tile([128, S // 2], bf16)      # odd K^T chunks at rows 64-127
            qt = kvp.tile([128, S // 2], bf16)      # Q^T, duplicated rows 64-127
            vn = kvp.tile([128, NKC, 65], bf16)     # V natural | ones, per chunk
            nc.vector.memset(vn[:, :, 64:65], 1.0)


            # ---- x tiles: quarter 0 split 16 ways across sync/scalar/
            # gpsimd queues for fastest arrival; later quarters issued up
            # front so transfers overlap compute ----
            xt = {}
            for qq in range(NQ):
                for d in range(DC):
                    xt[(qq, d)] = xtp.tile([128, 1024], bf16,
                                           tag=f"xt_{qq}_{d}", name=f"xt_{qq}_{d}")
            for d in range(DC):
                for h in range(2):
                    eng = nc.sync if h == 0 else nc.gpsimd
                    eng.dma_start(
                        out=xt[(0, d)][:, 512 * h:512 * (h + 1)],
                        in_=xT[128 * d:128 * (d + 1), 512 * h:512 * (h + 1)])

            def emit_x(qq):
                # next quarter's x, split across queues; scalar only for q1
                # (its ACT stream is idle until ~15us)
                engs = [nc.sync, nc.scalar, nc.gpsimd] if qq == 1 else                        [nc.sync, nc.gpsimd]
                for d in range(DC):
                    eng = engs[d % len(engs)]
                    eng.dma_start(
                        out=xt[(qq, d)][:],
                        in_=xT[128 * d:128 * (d + 1), 1024 * qq:1024 * (qq + 1)])

            # o_acc: groups t and t+2 share a PSUM buffer (t done before t+2)
            oacc = {}

            # scores-tile software pipeline state
            pend = [None]  # (ps, pt, pieces(list of (c, off, w, q_lo)), t)

            def flush_pv():
                if pend[0] is None:
                    return
                ps, pt, pieces, t, n_used = pend[0]
                for (c, off, w, q_lo) in pieces:
                    nc.tensor.matmul(
                        oacc[t][:, q_lo - 512 * t: q_lo - 512 * t + w],
                        vn[:, c, :], pt[:, off:off + w],
                        start=(c == 0), stop=(c >= 8 * t + 6),
                        skip_group_check=True)
                pend[0] = None

            def emit_tile(pieces, t):
                """pieces: [(c, off, w, q_lo)] with off in {0, 512} — one
                piece (= one matmul accumulation group) per 2KB PSUM bank."""
                ps = pscore.tile([128, 1024], f32, tag="sc", name="ps_sc")
                for (c, off, w, q_lo) in pieces:
                    if c % 2 == 0:
                        lhsT = kvt[0:64, 128 * c:128 * (c + 1)]
                        rhs = qt[0:64, q_lo:q_lo + w]
                    else:
                        lhsT = k2[64:128, 128 * (c // 2):128 * (c // 2 + 1)]
                        rhs = qt[64:128, q_lo:q_lo + w]
                    nc.tensor.matmul(ps[:, off:off + w], lhsT, rhs,
                                     start=True, stop=True)
                pt = ptp.tile([128, 1024], bf16, tag="pt", name="pt_t")
                exp = mybir.ActivationFunctionType.Exp
                if len(pieces) == 2 and pieces[0][2] == 512:
                    n = 512 + pieces[1][2]
                    nc.scalar.activation(pt[:, 0:n], ps[:, 0:n],
                                         func=exp, scale=SCALE)
                else:
                    for (c, off, w, q_lo) in pieces:
                        nc.scalar.activation(pt[:, off:off + w],
                                             ps[:, off:off + w],
                                             func=exp, scale=SCALE)
                # leading 128-col block of each chunk gets masked
                for (c, off, w, q_lo) in pieces:
                    if q_lo == 128 * (c // 2):
                        m = tril_sb if c % 2 == 0 else msk2_sb
                        nc.vector.tensor_mul(pt[:, off:off + 128],
                                             pt[:, off:off + 128], m[:, :])
                flush_pv()
                pend[0] = (ps, pt, pieces, t, 0)

            def emit_proj(qq):
                # ---- projections + per-quarter copies for quarter qq ----
                ps_q = pscore.tile([128, 1024], f32, tag="sc", name="ps_q")
                for d in range(DC):
                    rhs = xt[(qq, d)][:].rearrange(
                        "p (b e c) -> p e b c", e=2, c=128)[:, 0, :, :]
                    nc.tensor.matmul(ps_q[0:64, 0:512], wq_sb[:, d, :], rhs,
                                     start=(d == 0), stop=(d == DC - 1))
                nc.vector.tensor_scalar_add(qt[0:64, 512 * qq:512 * (qq + 1)],
                                            ps_q[0:64, 0:512], bq_sb[:, :])
                seng = nc.scalar if qq == 0 else nc.sync
                seng.dma_start(out=qt[64:128, 512 * qq:512 * (qq + 1)],
                               in_=qt[0:64, 512 * qq:512 * (qq + 1)])

                ps_kv = pscore.tile([128, 1024], f32, tag="sc", name="ps_kv")
                for sh in range(2):
                    for d in range(DC):
                        nc.tensor.matmul(
                            ps_kv[:, 512 * sh:512 * (sh + 1)], wkv_sb[:, d, :],
                            xt[(qq, d)][:, 512 * sh:512 * (sh + 1)],
                            start=(d == 0), stop=(d == DC - 1))
                nc.vector.tensor_scalar_add(
                    kvt[:, 1024 * qq:1024 * (qq + 1)], ps_kv[:, :],
                    bkv_sb[:, :])

                # odd K^T chunks -> partitions 64-127 (one strided DMA)
                ksrc = kvt[0:64, 1024 * qq:1024 * (qq + 1)].rearrange(
                    "p (k e c) -> p e k c", e=2, c=128)[:, 1, :, :]
                seng.dma_start(
                    out=k2[64:128, 512 * qq:512 * (qq + 1)], in_=ksrc)

                # V natural via PE transpose + DVE copy, per 128-chunk
                for lc in range(8):
                    c = 8 * qq + lc
                    ps_t = pscore.tile([128, 64], bf16, tag="tp", name="ps_t")
                    nc.tensor.transpose(
                        ps_t[:], kvt[64:128, 128 * c:128 * (c + 1)],
                        ident_bf[64:128, 64:128])
                    nc.vector.tensor_copy(out=vn[:, c, 0:64], in_=ps_t[:])

                if qq + 2 < NQ:
                    emit_x(qq + 2)

            # ---- PE warmup: each matmul consumes an arriving x q0 tile,
            # so the PE ramps its pstate exactly as the last tiles land ----
            ps_w = pscore.tile([128, 1024], f32, tag="sc")
            for i in range(N_WARMUP):
                nc.tensor.matmul(ps_w[:, 0:128], tril_sb[:],
                                 xt[(0, i % DC)][:, 0:128],
                                 start=True, stop=True)

            emit_x(1)
            emit_proj(0)
            for t in range(NQ):
                oacc[t] = pacc.tile([65, 512], f32, tag=f"oacc_{t % 2}",
                                    name=f"oacc_{t}")
                # one piece per 2KB PSUM bank (hw: one matmul accumulation
                # group per bank); consecutive (even, odd) chunk pairs per
                # tile give natural PE row-tiling
                nch = 8 * t + 8
                tiles = list(range(0, nch, 2))
                for ti, c0 in enumerate(tiles):
                    # next quarter's projections slot in 2 tiles before the
                    # group ends so the PE covers them while ACT drains
                    if t < NQ - 1 and ti == max(0, len(tiles) - 2):
                        emit_proj(t + 1)
                    pieces = []
                    for ci, off in ((c0, 0), (c0 + 1, 512)):
                        if ci < nch:
                            q_lo = max(512 * t, 128 * (ci // 2))
                            pieces.append((ci, off, 512 * (t + 1) - q_lo, q_lo))
                    emit_tile(pieces, t)
                flush_pv()

                # finalize this q-tile
                o_sb = osbp.tile([65, 512], f32, tag="osb")
                nc.vector.tensor_copy(out=o_sb[:], in_=oacc[t][:])
                nc.sync.dma_start(out=out[t, :, :], in_=o_sb[:])

    nc.compile()
    return nc


def get_nc():
    if "nc" not in _cached:
        _cached["nc"] = _build_nc()
    return _cached["nc"]


def prepare_in_maps(x, Wk, bk, Wq, bq, Wv, bv):
    wkv = np.ascontiguousarray(
        np.concatenate([Wk, Wv], axis=1).reshape(DC, 128, 128)
        .transpose(1, 0, 2)).astype(BF16)
    wq = np.ascontiguousarray(
        Wq.reshape(DC, 128, H).transpose(1, 0, 2)).astype(BF16)
    bkv = np.concatenate([bk, bv]).reshape(128, 1).astype(np.float32)
    bq_c = bq.reshape(H, 1).astype(np.float32)
    trilm = np.tril(np.ones((128, 128), np.float32)).T.astype(BF16)  # [kv,q]: q>=kv
    m2 = [np.zeros((128, 128), np.float32).astype(BF16),
          np.ones((128, 128), np.float32).astype(BF16)]

    swap = np.arange(NKC).reshape(-1, 2)[:, ::-1].reshape(-1)
    in_maps = []
    for core in range(N_CORES):
        b, j = core // 2, core % 2
        xTb = x[b].T                                        # (D, S)
        if j == 1:
            xTb = xTb.reshape(D, NKC, 128)[:, swap, :].reshape(D, S)
        in_maps.append({
            "xT": np.ascontiguousarray(xTb).astype(BF16),
            "wkv": wkv, "wq": wq, "bkv": bkv, "bq": bq_c,
            "tril": trilm, "msk2": m2[j],
        })
    return in_maps


def assemble_output(results):
    out = np.empty((B, S, H), dtype=np.float32)
    for core in range(N_CORES):
        b, j = core // 2, core % 2
        loc = results[core]["out"]                      # (NQ, 65, 512)
        o = loc[:, 0:64, :] / loc[:, 64:65, :]          # (NQ, H, 512)
        ob = o.reshape(NQ, H, 4, 128).transpose(0, 2, 3, 1)  # (t, bi, 128, H)
        full = out[b].reshape(NKC, 128, H)
        for t in range(NQ):
            for bi in range(4):
                full[8 * t + 2 * bi + j] = ob[t, bi]
    return out


def run_sharded(inputs, trace=False, trace_kwargs=None):
    from concourse.bass_utils import run_bass_kernel_spmd

    x = np.asarray(inputs["x"], dtype=np.float32)
    in_maps = prepare_in_maps(
        x,
        np.asarray(inputs["Wk"], dtype=np.float32),
        np.asarray(inputs["bk"], dtype=np.float32),
        np.asarray(inputs["Wq"], dtype=np.float32),
        np.asarray(inputs["bq"], dtype=np.float32),
        np.asarray(inputs["Wv"], dtype=np.float32),
        np.asarray(inputs["bv"], dtype=np.float32),
    )
    nc = get_nc()
    kw = {}
    if trace:
        kw["trace"] = True
        if trace_kwargs:
            kw.update(trace_kwargs)
    res = run_bass_kernel_spmd(nc, in_maps, core_ids=list(range(N_CORES)), **kw)
    return assemble_output(res.results), res


def kernel(**inputs):
    out, _ = run_sharded(inputs)
    return out


# revision 4
# speedup vs baseline: 1.0974x; 1.0974x over previous
"""Single-head causal attention (B=4, S=4096, D=1024, H=64) on 8 TRN2 NeuronCores.

v2: quarter-pipelined, causal-trimmed, ACT-rail-optimized.

Sharding: 2 cores per batch; core j of the pair takes q-blocks of parity j
(interleaved 128-row blocks balance causal work). Odd cores get x's kv
128-block pairs swapped host-side so the instruction stream is identical
(SPMD); the residual structural difference (odd-chunk leading block fully
masked for j=0, fully kept for j=1) is absorbed into a per-core mask tile.

Device algorithm per core (bf16 matmuls, f32 PSUM):
  per s-quarter qq (1024 kv positions, 512 q columns):
    K^T|V^T += [Wk|Wv]^T @ x^T   (8 d-chunks, 2x512-col segments)
    Q^T     = Wq^T @ x^T          (e=0 block-parity columns only)
    k2      = odd chunks of K^T copied to partitions 64-127 (row tiling)
    vn      = V natural per 128-chunk via DMA transpose, ones col appended
    group t=qq: for kv chunk c in 0..8t+7:
      piece = S^T(c, q in [max(512t, 128*(c//2)), 512(t+1)))  -- causal trim
      scores matmul into packed (128,1024) PSUM tiles; even chunks use
      kvt rows 0-63, odd chunks k2 rows 64-127 (concurrent row-tiled PE)
      one exp ACT per PSUM tile (scale=1/8 fused)
      leading 128-col block of every chunk masked: even chunks x tril,
      odd chunks x per-core 0/1 tile
      PV: o_acc[65, 512] += [V|1]^T @ P^T  (row 64 = softmax denominator)
  out[t] = o_acc_t  (host divides rows 0-63 by row 64 and scatters)
"""

import sys

for _p in ("/opt/trn_rl_repo", "/root/.axon_site"):
    if _p not in sys.path:
        sys.path.insert(0, _p)

import numpy as np
import ml_dtypes
import os
STAGE = int(os.environ.get("STAGE", "4"))
FEAT = int(os.environ.get("FEAT", "3"))  # 0=2mm+act 1=scores+ACT 2=+masks 3=+PV

B, S, D, H = 4, 4096, 1024, 64
N_CORES = 8
DC = D // 128           # 8 d-chunks
NQ = 4                  # s-quarters
NKC = S // 128          # 32 kv 128-blocks
SCALE = 1.0 / 8.0

BF16 = ml_dtypes.bfloat16

_cached = {}

N_WARMUP = 16           # 128-col PE warmup matmuls (~3.5us busy to raise HAM)


def _build_nc():
    from concourse import bacc, tile, mybir
    from concourse.masks import make_identity

    f32 = mybir.dt.float32
    bf16 = mybir.dt.bfloat16

    nc = bacc.Bacc("TRN2", target_bir_lowering=False, debug=False,
                   num_devices=N_CORES)

    xT = nc.declare_dram_parameter("xT", [D, S], bf16, isOutput=False)
    wkv = nc.declare_dram_parameter("wkv", [128, DC, 128], bf16, isOutput=False)
    wq = nc.declare_dram_parameter("wq", [128, DC, H], bf16, isOutput=False)
    bkv = nc.declare_dram_parameter("bkv", [128, 1], f32, isOutput=False)
    bqp = nc.declare_dram_parameter("bq", [H, 1], f32, isOutput=False)
    tril = nc.declare_dram_parameter("tril", [128, 128], bf16, isOutput=False)
    msk2 = nc.declare_dram_parameter("msk2", [128, 128], bf16, isOutput=False)
    out = nc.declare_dram_parameter("out", [NQ, 65, 512], f32, isOutput=True)

    with tile.TileContext(nc) as tc:
        with (
            tc.tile_pool(name="consts", bufs=1) as consts,
            tc.tile_pool(name="xtp", bufs=1) as xtp,
            tc.tile_pool(name="kvp", bufs=1) as kvp,
            tc.tile_pool(name="ptp", bufs=4) as ptp,
            tc.tile_pool(name="osbp", bufs=2) as osbp,
            tc.tile_pool(name="pscore", bufs=2, space="PSUM") as pscore,
            tc.tile_pool(name="pacc", bufs=1, space="PSUM") as pacc,
        ):
            # ---- constants on the scalar queue (idle until first exp);
            # sync/gpsimd queues are reserved for x streaming ----
            tril_sb = consts.tile([128, 128], bf16)
            nc.scalar.dma_start(out=tril_sb[:], in_=tril[:, :])
            msk2_sb = consts.tile([128, 128], bf16)
            nc.scalar.dma_start(out=msk2_sb[:], in_=msk2[:, :])
            bkv_sb = consts.tile([128, 1], f32)
            nc.scalar.dma_start(out=bkv_sb[:], in_=bkv[:, :])
            bq_sb = consts.tile([H, 1], f32)
            nc.scalar.dma_start(out=bq_sb[:], in_=bqp[:, :])
            wq_sb = consts.tile([128, DC, H], bf16)
            nc.scalar.dma_start(out=wq_sb[:], in_=wq[:, :, :])
            wkv_sb = consts.tile([128, DC, 128], bf16)
            nc.scalar.dma_start(out=wkv_sb[:, 0:4, :], in_=wkv[:, 0:4, :])
            nc.scalar.dma_start(out=wkv_sb[:, 4:8, :], in_=wkv[:, 4:8, :])
            ident_bf = consts.tile([128, 128], bf16)
            make_identity(nc, ident_bf[:, :])

            # ---- persistent SBUF tensors ----
            kvt = kvp.tile([128, S], bf16)          # rows 0-63 K^T, 64-127 V^T
            k2 = kvp.tile([128, S // 2], bf16)      # odd K^T chunks at rows 64-127
            qt = kvp.tile([128, S // 2], bf16)      # Q^T, duplicated rows 64-127
            vn = kvp.tile([128, NKC, 65], bf16)     # V natural | ones, per chunk
            nc.vector.memset(vn[:, :, 64:65], 1.0)


            # ---- x tiles: quarter 0 split 16 ways across sync/scalar/
            # gpsimd queues for fastest arrival; later quarters issued up
            # front so transfers overlap compute ----
            xt = {}
            for qq in range(NQ):
                for d in range(DC):
                    xt[(qq, d)] = xtp.tile([128, 1024], bf16,
                                           tag=f"xt_{qq}_{d}", name=f"xt_{qq}_{d}")
            for d in range(DC):
                for h in range(2):
                    eng = nc.sync if h == 0 else nc.gpsimd
                    eng.dma_start(
                        out=xt[(0, d)][:, 512 * h:512 * (h + 1)],
                        in_=xT[128 * d:128 * (d + 1), 512 * h:512 * (h + 1)])

            def emit_x(qq):
                # next quarter's x, split across queues; scalar only for q1
                # (its ACT stream is idle until ~15us)
                engs = [nc.sync, nc.scalar, nc.gpsimd] if qq == 1 else                        [nc.sync, nc.gpsimd]
                for d in range(DC):
                    eng = engs[d % len(engs)]
                    eng.dma_start(
                        out=xt[(qq, d)][:],
                        in_=xT[128 * d:128 * (d + 1), 1024 * qq:1024 * (qq + 1)])

            # o_acc: groups t and t+2 share a PSUM buffer (t done before t+2)
            oacc = {}

            # scores-tile software pipeline state
            pend = [None]  # (ps, pt, pieces(list of (c, off, w, q_lo)), t)

            def flush_pv():
                if pend[0] is None:
                    return
                ps, pt, pieces, t, n_used = pend[0]
                for (c, off, w, q_lo) in pieces:
                    nc.tensor.matmul(
                        oacc[t][:, q_lo - 512 * t: q_lo - 512 * t + w],
                        vn[:, c, :], pt[:, off:off + w],
                        start=(c == 0), stop=(c >= 8 * t + 6),
                        skip_group_check=True)
                pend[0] = None

            def emit_tile(pieces, t):
                """pieces: [(c, off, w, q_lo)] with off in {0, 512} — one
                piece (= one matmul accumulation group) per 2KB PSUM bank."""
                ps = pscore.tile([128, 1024], f32, tag="sc", name="ps_sc", bufs=3)
                for (c, off, w, q_lo) in pieces:
                    if c % 2 == 0:
                        lhsT = kvt[0:64, 128 * c:128 * (c + 1)]
                        rhs = qt[0:64, q_lo:q_lo + w]
                    else:
                        lhsT = k2[64:128, 128 * (c // 2):128 * (c // 2 + 1)]
                        rhs = qt[64:128, q_lo:q_lo + w]
                    nc.tensor.matmul(ps[:, off:off + w], lhsT, rhs,
                                     start=True, stop=True)
                pt = ptp.tile([128, 1024], bf16, tag="pt", name="pt_t")
                exp = mybir.ActivationFunctionType.Exp
                if len(pieces) == 2 and pieces[0][2] == 512:
                    n = 512 + pieces[1][2]
                    nc.scalar.activation(pt[:, 0:n], ps[:, 0:n],
                                         func=exp, scale=SCALE)
                else:
                    for (c, off, w, q_lo) in pieces:
                        nc.scalar.activation(pt[:, off:off + w],
                                             ps[:, off:off + w],
                                             func=exp, scale=SCALE)
                # leading 128-col block of each chunk gets masked
                for (c, off, w, q_lo) in pieces:
                    if q_lo == 128 * (c // 2):
                        m = tril_sb if c % 2 == 0 else msk2_sb
                        nc.vector.tensor_mul(pt[:, off:off + 128],
                                             pt[:, off:off + 128], m[:, :])
                flush_pv()
                pend[0] = (ps, pt, pieces, t, 0)

            def emit_proj_q(qq):
                ps_q = pscore.tile([128, 1024], f32, tag="sc", name="ps_q", bufs=3)
                for d in range(DC):
                    rhs = xt[(qq, d)][:].rearrange(
                        "p (b e c) -> p e b c", e=2, c=128)[:, 0, :, :]
                    nc.tensor.matmul(ps_q[0:64, 0:512], wq_sb[:, d, :], rhs,
                                     start=(d == 0), stop=(d == DC - 1))
                nc.vector.tensor_scalar_add(qt[0:64, 512 * qq:512 * (qq + 1)],
                                            ps_q[0:64, 0:512], bq_sb[:, :])
                seng = nc.scalar if qq == 0 else nc.sync
                seng.dma_start(out=qt[64:128, 512 * qq:512 * (qq + 1)],
                               in_=qt[0:64, 512 * qq:512 * (qq + 1)])

            def emit_proj_kv(qq):
                seng = nc.scalar if qq == 0 else nc.sync
                ps_kv = pscore.tile([128, 1024], f32, tag="sc", name="ps_kv", bufs=3)
                for sh in range(2):
                    for d in range(DC):
                        nc.tensor.matmul(
                            ps_kv[:, 512 * sh:512 * (sh + 1)], wkv_sb[:, d, :],
                            xt[(qq, d)][:, 512 * sh:512 * (sh + 1)],
                            start=(d == 0), stop=(d == DC - 1))
                nc.vector.tensor_scalar_add(
                    kvt[:, 1024 * qq:1024 * (qq + 1)], ps_kv[:, :],
                    bkv_sb[:, :])
                ksrc = kvt[0:64, 1024 * qq:1024 * (qq + 1)].rearrange(
                    "p (k e c) -> p e k c", e=2, c=128)[:, 1, :, :]
                seng.dma_start(
                    out=k2[64:128, 512 * qq:512 * (qq + 1)], in_=ksrc)
                for lc in range(8):
                    c = 8 * qq + lc
                    ps_t = pacc.tile([128, 64], bf16, tag="tp", name="ps_t")
                    nc.tensor.transpose(
                        ps_t[:], kvt[64:128, 128 * c:128 * (c + 1)],
                        ident_bf[64:128, 64:128])
                    nc.vector.tensor_copy(out=vn[:, c, 0:64], in_=ps_t[:])
                if qq + 2 < NQ:
                    emit_x(qq + 2)

            def emit_proj(qq):
                emit_proj_q(qq)
                emit_proj_kv(qq)

            emit_x(1)
            emit_proj(0)
            for t in range(NQ):
                oacc[t] = pacc.tile([65, 512], f32, tag="oacc",
                                    name=f"oacc_{t}")
                # one piece per 2KB PSUM bank (hw: one matmul accumulation
                # group per bank); consecutive (even, odd) chunk pairs per
                # tile give natural PE row-tiling
                nch = 8 * t + 8
                tiles = list(range(0, nch, 2))
                h1 = max(0, len(tiles) - 4)
                h2 = max(1, len(tiles) - 2)
                for ti, c0 in enumerate(tiles):
                    # next quarter's projections slot into the score stream
                    # in two halves so the ACT backlog absorbs each chunk
                    if t < NQ - 1 and ti == h1:
                        emit_proj_q(t + 1)
                    if t < NQ - 1 and ti == h2:
                        emit_proj_kv(t + 1)
                    pieces = []
                    for ci, off in ((c0, 0), (c0 + 1, 512)):
                        if ci < nch:
                            q_lo = max(512 * t, 128 * (ci // 2))
                            pieces.append((ci, off, 512 * (t + 1) - q_lo, q_lo))
                    emit_tile(pieces, t)
                flush_pv()

                # finalize this q-tile
                o_sb = osbp.tile([65, 512], f32, tag="osb")
                nc.vector.tensor_copy(out=o_sb[:], in_=oacc[t][:])
                nc.sync.dma_start(out=out[t, :, :], in_=o_sb[:])

    nc.compile()
    return nc


def get_nc():
    if "nc" not in _cached:
        _cached["nc"] = _build_nc()
    return _cached["nc"]


def prepare_in_maps(x, Wk, bk, Wq, bq, Wv, bv):
    wkv = np.ascontiguousarray(
        np.concatenate([Wk, Wv], axis=1).reshape(DC, 128, 128)
        .transpose(1, 0, 2)).astype(BF16)
    wq = np.ascontiguousarray(
        Wq.reshape(DC, 128, H).transpose(1, 0, 2)).astype(BF16)
    bkv = np.concatenate([bk, bv]).reshape(128, 1).astype(np.float32)
    bq_c = bq.reshape(H, 1).astype(np.float32)
    trilm = np.tril(np.ones((128, 128), np.float32)).T.astype(BF16)  # [kv,q]: q>=kv
    m2 = [np.zeros((128, 128), np.float32).astype(BF16),
          np.ones((128, 128), np.float32).astype(BF16)]

    swap = np.arange(NKC).reshape(-1, 2)[:, ::-1].reshape(-1)
    in_maps = []
    for core in range(N_CORES):
        b, j = core // 2, core % 2
        xTb = x[b].T                                        # (D, S)
        if j == 1:
            xTb = xTb.reshape(D, NKC, 128)[:, swap, :].reshape(D, S)
        in_maps.append({
            "xT": np.ascontiguousarray(xTb).astype(BF16),
            "wkv": wkv, "wq": wq, "bkv": bkv, "bq": bq_c,
            "tril": trilm, "msk2": m2[j],
        })
    return in_maps


def assemble_output(results):
    out = np.empty((B, S, H), dtype=np.float32)
    for core in range(N_CORES):
        b, j = core // 2, core % 2
        loc = results[core]["out"]                      # (NQ, 65, 512)
        o = loc[:, 0:64, :] / loc[:, 64:65, :]          # (NQ, H, 512)
        ob = o.reshape(NQ, H, 4, 128).transpose(0, 2, 3, 1)  # (t, bi, 128, H)
        full = out[b].reshape(NKC, 128, H)
        for t in range(NQ):
            for bi in range(4):
                full[8 * t + 2 * bi + j] = ob[t, bi]
    return out


def run_sharded(inputs, trace=False, trace_kwargs=None):
    from concourse.bass_utils import run_bass_kernel_spmd

    x = np.asarray(inputs["x"], dtype=np.float32)
    in_maps = prepare_in_maps(
        x,
        np.asarray(inputs["Wk"], dtype=np.float32),
        np.asarray(inputs["bk"], dtype=np.float32),
        np.asarray(inputs["Wq"], dtype=np.float32),
        np.asarray(inputs["bq"], dtype=np.float32),
        np.asarray(inputs["Wv"], dtype=np.float32),
        np.asarray(inputs["bv"], dtype=np.float32),
    )
    nc = get_nc()
    kw = {}
    if trace:
        kw["trace"] = True
        if trace_kwargs:
            kw.update(trace_kwargs)
    res = run_bass_kernel_spmd(nc, in_maps, core_ids=list(range(N_CORES)), **kw)
    return assemble_output(res.results), res


def kernel(**inputs):
    out, _ = run_sharded(inputs)
    return out


# revision 5
# speedup vs baseline: 1.1177x; 1.0185x over previous
"""Single-head causal attention (B=4, S=4096, D=1024, H=64) on 8 TRN2 NeuronCores.

v2: quarter-pipelined, causal-trimmed, ACT-rail-optimized.

Sharding: 2 cores per batch; core j of the pair takes q-blocks of parity j
(interleaved 128-row blocks balance causal work). Odd cores get x's kv
128-block pairs swapped host-side so the instruction stream is identical
(SPMD); the residual structural difference (odd-chunk leading block fully
masked for j=0, fully kept for j=1) is absorbed into a per-core mask tile.

Device algorithm per core (bf16 matmuls, f32 PSUM):
  per s-quarter qq (1024 kv positions, 512 q columns):
    K^T|V^T += [Wk|Wv]^T @ x^T   (8 d-chunks, 2x512-col segments)
    Q^T     = Wq^T @ x^T          (e=0 block-parity columns only)
    k2      = odd chunks of K^T copied to partitions 64-127 (row tiling)
    vn      = V natural per 128-chunk via DMA transpose, ones col appended
    group t=qq: for kv chunk c in 0..8t+7:
      piece = S^T(c, q in [max(512t, 128*(c//2)), 512(t+1)))  -- causal trim
      scores matmul into packed (128,1024) PSUM tiles; even chunks use
      kvt rows 0-63, odd chunks k2 rows 64-127 (concurrent row-tiled PE)
      one exp ACT per PSUM tile (scale=1/8 fused)
      leading 128-col block of every chunk masked: even chunks x tril,
      odd chunks x per-core 0/1 tile
      PV: o_acc[65, 512] += [V|1]^T @ P^T  (row 64 = softmax denominator)
  out[t] = o_acc_t  (host divides rows 0-63 by row 64 and scatters)
"""

import sys

for _p in ("/opt/trn_rl_repo", "/root/.axon_site"):
    if _p not in sys.path:
        sys.path.insert(0, _p)

import numpy as np
import ml_dtypes
import os
STAGE = int(os.environ.get("STAGE", "4"))
FEAT = int(os.environ.get("FEAT", "3"))  # 0=2mm+act 1=scores+ACT 2=+masks 3=+PV

B, S, D, H = 4, 4096, 1024, 64
N_CORES = 8
DC = D // 128           # 8 d-chunks
NQ = 4                  # s-quarters
NKC = S // 128          # 32 kv 128-blocks
SCALE = 1.0 / 8.0

BF16 = ml_dtypes.bfloat16

_cached = {}

N_WARMUP = 16           # 128-col PE warmup matmuls (~3.5us busy to raise HAM)


def _build_nc():
    from concourse import bacc, tile, mybir
    from concourse.masks import make_identity

    f32 = mybir.dt.float32
    bf16 = mybir.dt.bfloat16

    nc = bacc.Bacc("TRN2", target_bir_lowering=False, debug=False,
                   num_devices=N_CORES)

    xT = nc.declare_dram_parameter("xT", [D, S], bf16, isOutput=False)
    wkv = nc.declare_dram_parameter("wkv", [128, DC, 128], bf16, isOutput=False)
    wq = nc.declare_dram_parameter("wq", [128, DC, H], bf16, isOutput=False)
    bkv = nc.declare_dram_parameter("bkv", [128, 1], f32, isOutput=False)
    bqp = nc.declare_dram_parameter("bq", [H, 1], f32, isOutput=False)
    tril = nc.declare_dram_parameter("tril", [128, 128], bf16, isOutput=False)
    msk2 = nc.declare_dram_parameter("msk2", [128, 128], bf16, isOutput=False)
    out = nc.declare_dram_parameter("out", [NQ, 65, 512], f32, isOutput=True)

    with tile.TileContext(nc) as tc:
        with (
            tc.tile_pool(name="consts", bufs=1) as consts,
            tc.tile_pool(name="xtp", bufs=1) as xtp,
            tc.tile_pool(name="kvp", bufs=1) as kvp,
            tc.tile_pool(name="ptp", bufs=4) as ptp,
            tc.tile_pool(name="osbp", bufs=2) as osbp,
            tc.tile_pool(name="pscore", bufs=2, space="PSUM") as pscore,
            tc.tile_pool(name="pacc", bufs=1, space="PSUM") as pacc,
        ):
            # ---- constants on the scalar queue (idle until first exp);
            # sync/gpsimd queues are reserved for x streaming ----
            tril_sb = consts.tile([128, 128], bf16)
            nc.scalar.dma_start(out=tril_sb[:], in_=tril[:, :])
            msk2_sb = consts.tile([128, 128], bf16)
            nc.scalar.dma_start(out=msk2_sb[:], in_=msk2[:, :])
            bkv_sb = consts.tile([128, 1], f32)
            nc.scalar.dma_start(out=bkv_sb[:], in_=bkv[:, :])
            bq_sb = consts.tile([H, 1], f32)
            nc.scalar.dma_start(out=bq_sb[:], in_=bqp[:, :])
            wq_sb = consts.tile([128, DC, H], bf16)
            nc.scalar.dma_start(out=wq_sb[:], in_=wq[:, :, :])
            wkv_sb = consts.tile([128, DC, 128], bf16)
            nc.scalar.dma_start(out=wkv_sb[:, 0:4, :], in_=wkv[:, 0:4, :])
            nc.scalar.dma_start(out=wkv_sb[:, 4:8, :], in_=wkv[:, 4:8, :])
            ident_bf = consts.tile([128, 128], bf16)
            make_identity(nc, ident_bf[:, :])

            # ---- persistent SBUF tensors ----
            kvt = kvp.tile([128, S], bf16)          # rows 0-63 K^T, 64-127 V^T
            k2 = kvp.tile([128, S // 2], bf16)      # odd K^T chunks at rows 64-127
            qt = kvp.tile([128, S // 2], bf16)      # Q^T, duplicated rows 64-127
            vn = kvp.tile([128, NKC, 65], bf16)     # V natural | ones, per chunk
            nc.vector.memset(vn[:, :, 64:65], 1.0)


            # ---- x tiles: quarter 0 split 16 ways across sync/scalar/
            # gpsimd queues for fastest arrival; later quarters issued up
            # front so transfers overlap compute ----
            xt = {}
            for qq in range(NQ):
                for d in range(DC):
                    xt[(qq, d)] = xtp.tile([128, 1024], bf16,
                                           tag=f"xt_{qq}_{d}", name=f"xt_{qq}_{d}")
            for d in range(DC):
                for h in range(2):
                    eng = nc.sync if h == 0 else nc.gpsimd
                    eng.dma_start(
                        out=xt[(0, d)][:, 512 * h:512 * (h + 1)],
                        in_=xT[128 * d:128 * (d + 1), 512 * h:512 * (h + 1)])

            def emit_x(qq):
                # next quarter's x, split across queues; scalar only for q1
                # (its ACT stream is idle until ~15us)
                engs = [nc.sync, nc.scalar, nc.gpsimd] if qq == 1 else                        [nc.sync, nc.gpsimd]
                for d in range(DC):
                    eng = engs[d % len(engs)]
                    eng.dma_start(
                        out=xt[(qq, d)][:],
                        in_=xT[128 * d:128 * (d + 1), 1024 * qq:1024 * (qq + 1)])

            # o_acc: groups t and t+2 share a PSUM buffer (t done before t+2)
            oacc = {}

            # scores-tile software pipeline state
            pend = [None]  # (ps, pt, pieces(list of (c, off, w, q_lo)), t)

            def flush_pv():
                if pend[0] is None:
                    return
                ps, pt, pieces, t, n_used = pend[0]
                for (c, off, w, q_lo) in pieces:
                    nc.tensor.matmul(
                        oacc[t][:, q_lo - 512 * t: q_lo - 512 * t + w],
                        vn[:, c, :], pt[:, off:off + w],
                        start=(c == 0), stop=(c >= 8 * t + 6),
                        skip_group_check=True)
                pend[0] = None

            def emit_tile(pieces, t):
                """pieces: [(c, off, w, q_lo)] with off in {0, 512} — one
                piece (= one matmul accumulation group) per 2KB PSUM bank."""
                ps = pscore.tile([128, 1024], f32, tag="sc", name="ps_sc", bufs=3)
                for (c, off, w, q_lo) in pieces:
                    if c % 2 == 0:
                        lhsT = kvt[0:64, 128 * c:128 * (c + 1)]
                        rhs = qt[0:64, q_lo:q_lo + w]
                    else:
                        lhsT = k2[64:128, 128 * (c // 2):128 * (c // 2 + 1)]
                        rhs = qt[64:128, q_lo:q_lo + w]
                    nc.tensor.matmul(ps[:, off:off + w], lhsT, rhs,
                                     start=True, stop=True)
                pt = ptp.tile([128, 1024], bf16, tag="pt", name="pt_t")
                exp = mybir.ActivationFunctionType.Exp
                if len(pieces) == 2 and pieces[0][2] == 512:
                    n = 512 + pieces[1][2]
                    nc.scalar.activation(pt[:, 0:n], ps[:, 0:n],
                                         func=exp, scale=SCALE)
                else:
                    for (c, off, w, q_lo) in pieces:
                        nc.scalar.activation(pt[:, off:off + w],
                                             ps[:, off:off + w],
                                             func=exp, scale=SCALE)
                # leading 128-col block of each chunk gets masked
                for (c, off, w, q_lo) in pieces:
                    if q_lo == 128 * (c // 2):
                        m = tril_sb if c % 2 == 0 else msk2_sb
                        nc.vector.tensor_mul(pt[:, off:off + 128],
                                             pt[:, off:off + 128], m[:, :])
                flush_pv()
                pend[0] = (ps, pt, pieces, t, 0)

            def proj_parts(qq):
                """Next-quarter projections as ~2-3us parts, interleaved
                between score tiles so the ACT backlog absorbs each."""
                st = {}

                def p1():
                    st["q"] = pscore.tile([128, 1024], f32, tag="sc",
                                          name="ps_q", bufs=3)
                    for d in range(DC):
                        rhs = xt[(qq, d)][:].rearrange(
                            "p (b e c) -> p e b c", e=2, c=128)[:, 0, :, :]
                        nc.tensor.matmul(st["q"][0:64, 0:512], wq_sb[:, d, :],
                                         rhs, start=(d == 0),
                                         stop=(d == DC - 1))
                    nc.vector.tensor_scalar_add(
                        qt[0:64, 512 * qq:512 * (qq + 1)],
                        st["q"][0:64, 0:512], bq_sb[:, :])
                    seng = nc.scalar if qq == 0 else nc.sync
                    seng.dma_start(out=qt[64:128, 512 * qq:512 * (qq + 1)],
                                   in_=qt[0:64, 512 * qq:512 * (qq + 1)])

                def p2():
                    st["kv"] = pscore.tile([128, 1024], f32, tag="sc",
                                           name="ps_kv", bufs=3)
                    for d in range(DC):
                        nc.tensor.matmul(
                            st["kv"][:, 0:512], wkv_sb[:, d, :],
                            xt[(qq, d)][:, 0:512],
                            start=(d == 0), stop=(d == DC - 1))

                def p3():
                    for d in range(DC):
                        nc.tensor.matmul(
                            st["kv"][:, 512:1024], wkv_sb[:, d, :],
                            xt[(qq, d)][:, 512:1024],
                            start=(d == 0), stop=(d == DC - 1))
                    nc.vector.tensor_scalar_add(
                        kvt[:, 1024 * qq:1024 * (qq + 1)], st["kv"][:, :],
                        bkv_sb[:, :])
                    seng = nc.scalar if qq == 0 else nc.sync
                    ksrc = kvt[0:64, 1024 * qq:1024 * (qq + 1)].rearrange(
                        "p (k e c) -> p e k c", e=2, c=128)[:, 1, :, :]
                    seng.dma_start(
                        out=k2[64:128, 512 * qq:512 * (qq + 1)], in_=ksrc)

                def p4():
                    for lc in range(8):
                        c = 8 * qq + lc
                        ps_t = pacc.tile([128, 64], bf16, tag="tp",
                                         name="ps_t")
                        nc.tensor.transpose(
                            ps_t[:], kvt[64:128, 128 * c:128 * (c + 1)],
                            ident_bf[64:128, 64:128])
                        nc.vector.tensor_copy(out=vn[:, c, 0:64], in_=ps_t[:])
                    if qq + 2 < NQ:
                        emit_x(qq + 2)

                return [p1, p2, p3, p4]

            def emit_proj(qq):
                for p in proj_parts(qq):
                    p()

            emit_x(1)
            emit_proj(0)
            for t in range(NQ):
                oacc[t] = pacc.tile([65, 512], f32, tag="oacc",
                                    name=f"oacc_{t}")
                # one piece per 2KB PSUM bank (hw: one matmul accumulation
                # group per bank); consecutive (even, odd) chunk pairs per
                # tile give natural PE row-tiling
                nch = 8 * t + 8
                tiles = list(range(0, nch, 2))
                parts = proj_parts(t + 1) if t < NQ - 1 else []
                for ti, c0 in enumerate(tiles):
                    # next quarter's projections trickle into the score
                    # stream so the ACT backlog absorbs each part
                    while parts and len(parts) > len(tiles) - 1 - ti:
                        parts.pop(0)()
                    pieces = []
                    for ci, off in ((c0, 0), (c0 + 1, 512)):
                        if ci < nch:
                            q_lo = max(512 * t, 128 * (ci // 2))
                            pieces.append((ci, off, 512 * (t + 1) - q_lo, q_lo))
                    emit_tile(pieces, t)
                flush_pv()

                # finalize this q-tile
                o_sb = osbp.tile([65, 512], f32, tag="osb")
                nc.vector.tensor_copy(out=o_sb[:], in_=oacc[t][:])
                nc.sync.dma_start(out=out[t, :, :], in_=o_sb[:])

    nc.compile()
    return nc


def get_nc():
    if "nc" not in _cached:
        _cached["nc"] = _build_nc()
    return _cached["nc"]


def prepare_in_maps(x, Wk, bk, Wq, bq, Wv, bv):
    wkv = np.ascontiguousarray(
        np.concatenate([Wk, Wv], axis=1).reshape(DC, 128, 128)
        .transpose(1, 0, 2)).astype(BF16)
    wq = np.ascontiguousarray(
        Wq.reshape(DC, 128, H).transpose(1, 0, 2)).astype(BF16)
    bkv = np.concatenate([bk, bv]).reshape(128, 1).astype(np.float32)
    bq_c = bq.reshape(H, 1).astype(np.float32)
    trilm = np.tril(np.ones((128, 128), np.float32)).T.astype(BF16)  # [kv,q]: q>=kv
    m2 = [np.zeros((128, 128), np.float32).astype(BF16),
          np.ones((128, 128), np.float32).astype(BF16)]

    swap = np.arange(NKC).reshape(-1, 2)[:, ::-1].reshape(-1)
    in_maps = []
    for core in range(N_CORES):
        b, j = core // 2, core % 2
        xTb = x[b].T                                        # (D, S)
        if j == 1:
            xTb = xTb.reshape(D, NKC, 128)[:, swap, :].reshape(D, S)
        in_maps.append({
            "xT": np.ascontiguousarray(xTb).astype(BF16),
            "wkv": wkv, "wq": wq, "bkv": bkv, "bq": bq_c,
            "tril": trilm, "msk2": m2[j],
        })
    return in_maps


def assemble_output(results):
    out = np.empty((B, S, H), dtype=np.float32)
    for core in range(N_CORES):
        b, j = core // 2, core % 2
        loc = results[core]["out"]                      # (NQ, 65, 512)
        o = loc[:, 0:64, :] / loc[:, 64:65, :]          # (NQ, H, 512)
        ob = o.reshape(NQ, H, 4, 128).transpose(0, 2, 3, 1)  # (t, bi, 128, H)
        full = out[b].reshape(NKC, 128, H)
        for t in range(NQ):
            for bi in range(4):
                full[8 * t + 2 * bi + j] = ob[t, bi]
    return out


def run_sharded(inputs, trace=False, trace_kwargs=None):
    from concourse.bass_utils import run_bass_kernel_spmd

    x = np.asarray(inputs["x"], dtype=np.float32)
    in_maps = prepare_in_maps(
        x,
        np.asarray(inputs["Wk"], dtype=np.float32),
        np.asarray(inputs["bk"], dtype=np.float32),
        np.asarray(inputs["Wq"], dtype=np.float32),
        np.asarray(inputs["bq"], dtype=np.float32),
        np.asarray(inputs["Wv"], dtype=np.float32),
        np.asarray(inputs["bv"], dtype=np.float32),
    )
    nc = get_nc()
    kw = {}
    if trace:
        kw["trace"] = True
        if trace_kwargs:
            kw.update(trace_kwargs)
    res = run_bass_kernel_spmd(nc, in_maps, core_ids=list(range(N_CORES)), **kw)
    return assemble_output(res.results), res


def kernel(**inputs):
    out, _ = run_sharded(inputs)
    return out
